# revision 1
# baseline (speedup 1.0000x reference)
"""Deformable Conv1D on 8 Trainium2 NeuronCores (Bass/Tile).

Math (reference): out[b,o,l] = sum_{i,k} W[o,i,k] * interp[b,i,l,k] + bias[o]
  interp[b,i,l,k] = wa*x[b,i,x0c] + wb*x[b,i,x1c],  loc = l + k + off[b,l,k]
  x0c/x1c = clip(floor(loc))/clip(floor(loc)+1), wa = x1c-loc, wb = loc-x0c.

Device decomposition per core (core j: batch b=j//2, L-half S=4096*(j%2)):
  Phase 1 (PE): Y_k^T[t, o] = sum_i x[b,i,t] * W[o,i,k]   (matmul, fp32r)
  Phase 2 (PE): out^T[l, o] = sum_k sum_t G_k[t, l] * Y_k^T[t, o]
    where G_k is a host-built banded selector holding the interpolation
    weights wa/wb at rows t = x0c/x1c (offsets are data-dependent but small:
    |floor(off)| <= 4, so a 128-row band covers a 113-wide output chunk).
  Host does: the tiny offset conv (2.7% of FLOPs), G assembly (pure
  addressing), and the final [l,o] -> [o,l] transpose.

All heavy FLOPs (30.1 GFLOP of matmul) run on the PE engines of 8 cores.
"""

import numpy as np

import concourse.bacc as bacc
import concourse.bass as bass
import concourse.mybir as mybir
import concourse.tile as tile
from concourse.bass_utils import run_bass_kernel_spmd

# Problem constants (hardcoded per harness contract).
B, CIN, COUT, L = 4, 256, 256, 8192
K, PAD = 7, 3
NCORE = 8
HALF = L // 2              # 4096 output positions per core
CHUNK = 113                # output positions per window (band 128 covers s in [-4,4])
NWIN = -(-HALF // CHUNK)   # 37
XPW = 4224                 # padded x width per core (needs 113*36+128 = 4196)
HALO = 4                   # x_pad global col 0 == S - HALO
F32 = mybir.dt.float32
F32R = mybir.dt.float32r


def _np_dt(qdt):
    if qdt == mybir.dt.bfloat16:
        import ml_dtypes
        return ml_dtypes.bfloat16
    if qdt == mybir.dt.float16:
        return np.float16
    return np.float32

# Matmul operand dtype: float16 streams 1 col/cycle on the PE (fp32/fp32r
# moving operands stream at half rate) and halves DMA traffic, with a
# 10-bit mantissa (rel err ~1e-3 end to end vs 3e-4 for fp32r).
QDT = mybir.dt.float16
GCOLS = CHUNK if QDT == mybir.dt.float32r else 128   # pad to 128 for FWL
_NC_CACHE = {}


def _build_nc(passes=1):
    key = ("nc", passes)
    if key in _NC_CACHE:
        return _NC_CACHE[key]
    qdt = QDT
    nc = bacc.Bacc("TRN2", target_bir_lowering=False, debug=False, num_devices=NCORE)
    x_d = nc.dram_tensor("xp", [2, 128, XPW], QDT, kind="ExternalInput")
    w_d = nc.dram_tensor("wt", [2, K, 128, COUT], QDT, kind="ExternalInput")
    g_d = nc.dram_tensor("gsel", [NWIN, 128, K, GCOLS], qdt, kind="ExternalInput")
    b_d = nc.dram_tensor("bias", [1, COUT], F32, kind="ExternalInput")
    o_d = nc.dram_tensor("out", [HALF, COUT], F32, kind="ExternalOutput")

    with tile.TileContext(nc) as tc:
        with (
            tc.tile_pool(name="const", bufs=1) as cpool,
            tc.tile_pool(name="gp", bufs=3) as gpool,
            tc.tile_pool(name="yp", bufs=2) as ypool,
            tc.tile_pool(name="op", bufs=3) as opool,
            tc.tile_pool(name="ps1", bufs=7, space="PSUM") as ps1,
            tc.tile_pool(name="ps2", bufs=1, space="PSUM") as ps2,
        ):
            # ---- constants: x halves, weights, bias tile ----
            x_sb = []
            for i in range(2):
                xt = cpool.tile([128, XPW], QDT, tag=f"x{i}")
                nc.sync.dma_start(xt[:], x_d[i])
                x_sb.append(xt)
            w_sb = cpool.tile([128, 2, K, COUT], QDT, tag="w")
            nc.sync.dma_start(w_sb[:], w_d.rearrange("i k p o -> p i k o"))
            bias_row = cpool.tile([1, COUT], F32, tag="br")
            nc.sync.dma_start(bias_row[:], b_d[:])
            ones_col = cpool.tile([1, CHUNK], F32, tag="oc")
            nc.vector.memset(ones_col[:], 1.0)
            bias_ps = ps2.tile([CHUNK, COUT], F32, tag="ops")
            nc.tensor.matmul(bias_ps[:], ones_col[:], bias_row[:], start=True, stop=True)
            bias_tile = cpool.tile([CHUNK, COUT], F32, tag="bt")
            nc.vector.tensor_copy(bias_tile[:], bias_ps[:])

            # ---- software-pipelined window loop ----
            state = {}  # window ci -> (g_tile, [y_k tiles])

            def phase1(ci):
                gt = gpool.tile([128, K, GCOLS], qdt, tag="g")
                nc.sync.dma_start(gt[:], g_d[ci])
                yps = [ps1.tile([128, COUT], F32, tag="yps", name=f"yps{k}")
                       for k in range(K)]
                for i in range(2):
                    lhs = x_sb[i][:, CHUNK * ci:CHUNK * ci + 128]
                    for k in range(K):
                        nc.tensor.matmul(yps[k][:], lhs, w_sb[:, i, k, :],
                                         start=(i == 0), stop=(i == 1))
                ys = []
                for k in range(K):
                    yt = ypool.tile([128, COUT], qdt, tag=f"y{k}", name=f"y{k}")
                    eng = nc.vector if k % 2 == 0 else nc.scalar
                    if eng is nc.vector:
                        nc.vector.tensor_copy(yt[:], yps[k][:])
                    else:
                        nc.scalar.copy(yt[:], yps[k][:])
                    ys.append(yt)
                state[ci] = (gt, ys)

            def phase2(ci):
                gt, ys = state.pop(ci)
                ops = ps2.tile([GCOLS, COUT], F32, tag="ops")
                for k in range(K):
                    nc.tensor.matmul(ops[:], gt[:, k, :], ys[k][:],
                                     start=(k == 0), stop=(k == K - 1))
                osb = opool.tile([CHUNK, COUT], F32, tag="o")
                nc.vector.tensor_add(osb[:], ops[:CHUNK, :], bias_tile[:])
                rows = min(CHUNK, HALF - CHUNK * ci)
                nc.sync.dma_start(o_d[CHUNK * ci:CHUNK * ci + rows, :], osb[:rows, :])

            for rep in range(passes):
                for ci in range(NWIN):
                    phase1(ci)
                    if ci > 0 or rep > 0:
                        phase2((ci - 1) % NWIN)
            phase2(NWIN - 1)

    nc.finalize()
    _NC_CACHE[key] = nc
    return nc


def _host_prep(x, weight, bias, offset_w, offset_b):
    """Offset conv + selector build on host. Returns per-core input maps."""
    x = np.ascontiguousarray(x, np.float32)
    weight = np.asarray(weight, np.float32)
    bias = np.asarray(bias, np.float32)
    offset_w = np.asarray(offset_w, np.float32)
    offset_b = np.asarray(offset_b, np.float32)

    # offsets[b, kk, l] (same math as reference conv, fp32)
    xpc = np.zeros((B, CIN, L + 2 * PAD), np.float32)
    xpc[:, :, PAD:PAD + L] = x
    offs = np.zeros((B, K, L), np.float32)
    for k2 in range(K):
        offs += np.einsum("kc,bcl->bkl", offset_w[:, :, k2],
                          xpc[:, :, k2:k2 + L], optimize=True)
    offs += offset_b[None, :, None]

    # loc per (b, l, k); p + p_k + PAD == l + k exactly in fp32
    lk = (np.arange(L, dtype=np.float32)[:, None]
          + np.arange(K, dtype=np.float32)[None, :])      # [L, K]
    loc = lk[None] + np.transpose(offs, (0, 2, 1))        # [B, L, K]
    x0 = np.floor(loc).astype(np.int64)
    x0c = np.clip(x0, 0, L - 1)
    x1c = np.clip(x0 + 1, 0, L - 1)
    wa = x1c.astype(np.float32) - loc
    wb = loc - x0c.astype(np.float32)

    wt = np.ascontiguousarray(
        weight.reshape(COUT, 2, 128, K).transpose(1, 3, 2, 0))  # [i,k,p,o]
    bias_row = bias.reshape(1, COUT)

    npq = _np_dt(QDT)
    in_maps = []
    for core in range(NCORE):
        b, half = divmod(core, 2)
        S = HALF * half
        # x_pad: global cols [S-HALO, S-HALO+XPW)
        xp = np.zeros((CIN, XPW), np.float32)
        lo, hi = S - HALO, S - HALO + XPW
        cl, ch = max(0, lo), min(L, hi)
        xp[:, cl - lo:ch - lo] = x[b, :, cl:ch]

        # selector G[ci, k, u, q]
        G = np.zeros((NWIN, K, 128, GCOLS), np.float32)
        l_idx = S + np.arange(HALF)                      # global l for q-slots
        ci = np.arange(HALF) // CHUNK
        q = np.arange(HALF) % CHUNK
        band0 = (S + ci * CHUNK - HALO)                  # global band start
        for k in range(K):
            u0 = x0c[b, l_idx, k] - band0
            u1 = x1c[b, l_idx, k] - band0
            if u0.min() < 0 or u1.max() > 127:
                raise AssertionError(
                    f"offset out of band: u0min={u0.min()} u1max={u1.max()}")
            flat = G.reshape(-1)
            base = ((ci * K + k) * 128 + u0) * GCOLS + q
            np.add.at(flat, base, wa[b, l_idx, k])
            base = ((ci * K + k) * 128 + u1) * GCOLS + q
            np.add.at(flat, base, wb[b, l_idx, k])

        in_maps.append({
            "xp": np.ascontiguousarray(xp.reshape(2, 128, XPW)).astype(npq),
            "wt": wt.astype(npq),
            "gsel": np.ascontiguousarray(G.transpose(0, 2, 1, 3)).astype(npq),
            "bias": bias_row,
        })
    return in_maps


def _assemble(results):
    out = np.empty((B, COUT, L), np.float32)
    for b in range(B):
        lo_half = results[2 * b]["out"]       # [4096, 256] rows l in [0,4096)
        hi_half = results[2 * b + 1]["out"]
        out[b, :, :HALF] = lo_half.T
        out[b, :, HALF:] = hi_half.T
    return out


def kernel(x, weight, bias, offset_w, offset_b):
    nc = _build_nc()
    in_maps = _host_prep(x, weight, bias, offset_w, offset_b)
    res = run_bass_kernel_spmd(nc, in_maps, core_ids=list(range(NCORE)))
    return _assemble(res.results)


def kernel_timed(inputs, repeats=3):
    """Dev helper: returns (out, wall_times_s per run)."""
    import time
    nc = _build_nc()
    in_maps = _host_prep(**inputs)
    times, res = [], None
    for _ in range(repeats):
        t0 = time.time()
        res = run_bass_kernel_spmd(nc, in_maps, core_ids=list(range(NCORE)))
        times.append(time.time() - t0)
    return _assemble(res.results), times



# revision 3
# speedup vs baseline: 3.5577x; 3.5577x over previous
"""Deformable Conv1D on 8 Trainium2 NeuronCores (Bass/Tile), axon-tunneled.

Math (reference): out[b,o,l] = sum_{i,k} W[o,i,k] * interp[b,i,l,k] + bias[o]
  interp[b,i,l,k] = lerp of x[b,i,:] at loc = l + k + off[b,l,k], with
  torch-style clamped endpoints (which make any sample with
  loc outside [0, L-1) contribute exactly zero).

Device decomposition per core (core j: batch b=j//2, L-half S=4096*(j%2)):
  Phase 0 (PE+ACT+DVE+Pool): offset conv  off[q,k] (matmul over Cin,K),
    then selector G_k[u,q] = valid(loc) * max(0, 1 - |loc_rel - u|)
    (the lerp-with-clamp is exactly a tent function; clipped samples are
    zero).  Built transposed via per-partition-scalar vector ops, then
    PE-transposed into lhsT layout.
  Phase 1 (PE): Y_k^T[t,o] = sum_i x[b,i,t] * W[o,i,k]   (fp16 matmul)
  Phase 2 (PE): out^T[l,o] = sum_k sum_t G_k[t,l] * Y_k^T[t,o]

Everything is computed on device; the tunnel only moves x (fp16, sharded),
the small weights (replicated), and the fp16 output back.  The jitted
PJRT callable is built once and cached (run_bass_kernel_spmd's axon path
rebuilds it per call, which costs ~0.7s/call in retrace overhead).
"""

import numpy as np

import jax
import jax.numpy as jnp
from jax.sharding import Mesh, PartitionSpec, NamedSharding
from jax.experimental.shard_map import shard_map

import concourse.bacc as bacc
import concourse.bass as bass
import concourse.mybir as mybir
import concourse.tile as tile
import concourse.bass2jax as b2j
from concourse.masks import make_identity

# Problem constants (hardcoded per harness contract).
B, CIN, COUT, L = 4, 256, 256, 8192
K, PAD = 7, 3
NCORE = 8
HALF = L // 2              # 4096 output positions per core
CHUNK = 113                # output positions per window (128-row band covers it)
NWIN = -(-HALF // CHUNK)   # 37
XPW = 4224                 # padded x width per core
HALO = 4                   # x_pad global col 0 == S - HALO
F32 = mybir.dt.float32
F16 = mybir.dt.float16
I32 = mybir.dt.int32
OP = mybir.AluOpType
ACT = mybir.ActivationFunctionType

MASKED_WINS = (0, NWIN - 1)   # only these can have out-of-range samples

_CACHE = {}


def _build_nc():
    nc = bacc.Bacc("TRN2", target_bir_lowering=False, debug=False,
                   num_devices=NCORE)
    x_d = nc.dram_tensor("xp", [2, 128, XPW], F16, kind="ExternalInput")
    w_d = nc.dram_tensor("wt", [2, K, 128, COUT], F16, kind="ExternalInput")
    ow_d = nc.dram_tensor("ow", [2, K, 128, K], F16, kind="ExternalInput")
    ck_d = nc.dram_tensor("ck", [128, K], F32, kind="ExternalInput")
    bnd_d = nc.dram_tensor("bnd", [128, 4 * K], F32, kind="ExternalInput")
    b_d = nc.dram_tensor("bias", [1, COUT], F32, kind="ExternalInput")
    o_d = nc.dram_tensor("out", [HALF, COUT], F16, kind="ExternalOutput")

    with tile.TileContext(nc) as tc:
        with (
            tc.tile_pool(name="const", bufs=1) as cpool,
            tc.tile_pool(name="nlp", bufs=2) as nlpool,
            tc.tile_pool(name="dabp", bufs=3) as dabpool,
            tc.tile_pool(name="ttp", bufs=3) as ttpool,
            tc.tile_pool(name="vnp", bufs=4) as vnpool,
            tc.tile_pool(name="gtp", bufs=3) as gtpool,
            tc.tile_pool(name="gsbp", bufs=2) as gsbpool,
            tc.tile_pool(name="ysp", bufs=2) as yspool,
            tc.tile_pool(name="osp", bufs=3) as ospool,
            tc.tile_pool(name="psoff", bufs=1, space="PSUM") as psoff,
            tc.tile_pool(name="psy", bufs=3, space="PSUM") as psy,
            tc.tile_pool(name="pstr", bufs=2, space="PSUM") as pstr,
            tc.tile_pool(name="pso", bufs=2, space="PSUM") as pso,
        ):
            # ---- constants ----
            x_sb = []
            for i in range(2):
                xt = cpool.tile([128, XPW], F16, tag=f"x{i}")
                nc.sync.dma_start(xt[:], x_d[i])
                x_sb.append(xt)
            w_sb = cpool.tile([128, 2, K, COUT], F16, tag="w")
            nc.sync.dma_start(w_sb[:], w_d.rearrange("i k p o -> p i k o"))
            ow_sb = cpool.tile([128, 2, K, K], F16, tag="ow")
            nc.sync.dma_start(ow_sb[:], ow_d.rearrange("i k p o -> p i k o"))
            ck_sb = cpool.tile([128, K], F32, tag="ck")
            nc.sync.dma_start(ck_sb[:], ck_d[:])
            bnd_sb = cpool.tile([128, 4 * K], F32, tag="bnd")
            nc.sync.dma_start(bnd_sb[:], bnd_d[:])
            bias_row = cpool.tile([1, COUT], F32, tag="br")
            nc.sync.dma_start(bias_row[:], b_d[:])

            ident = cpool.tile([128, 128], F16, tag="id")
            make_identity(nc, ident)

            # iota tiles: u along free (all partitions identical), q down parts
            iota_i = cpool.tile([128, 128], I32, tag="ioti")
            nc.gpsimd.iota(iota_i[:], pattern=[[1, 128]], base=0,
                           channel_multiplier=0)
            iota_f = cpool.tile([128, 128], F32, tag="iotf")
            nc.vector.tensor_copy(iota_f[:], iota_i[:])
            qi_i = cpool.tile([128, 1], I32, tag="qii")
            nc.gpsimd.iota(qi_i[:], pattern=[[1, 1]], base=0,
                           channel_multiplier=1)
            qi_f = cpool.tile([128, 1], F32, tag="qif")
            nc.vector.tensor_copy(qi_f[:], qi_i[:])
            # iota_km[k][u] = u - crow[k]
            iota_km = cpool.tile([128, K, 128], F32, tag="iokm")
            for k in range(K):
                nc.vector.tensor_scalar(iota_km[:, k, :], iota_f[:],
                                        ck_sb[:, k:k + 1], None, OP.subtract)

            # bias tile [128, COUT] via ones-broadcast matmul
            ones_col = cpool.tile([1, 128], F32, tag="oc")
            nc.vector.memset(ones_col[:], 1.0)
            bias_ps = pso.tile([128, COUT], F32, tag="ops")
            nc.tensor.matmul(bias_ps[:], ones_col[:], bias_row[:],
                             start=True, stop=True)
            bias_sb = cpool.tile([128, COUT], F32, tag="bt")
            nc.vector.tensor_copy(bias_sb[:], bias_ps[:])

            # ---- window loop ----
            for ci in range(NWIN):
                a0 = CHUNK * ci          # xp col of band row u=0
                # Phase 0a: offset conv -> psum[q,k] (no +q term yet)
                offp = psoff.tile([128, K], F32, tag="offp")
                n = 0
                for i in range(2):
                    for k2 in range(K):
                        lhs = x_sb[i][:, a0 + 1 + k2:a0 + 1 + k2 + 128]
                        nc.tensor.matmul(offp[:], lhs, ow_sb[:, i, k2, :],
                                         start=(n == 0), stop=(n == 13))
                        n += 1
                # nloc[q,k] = -(conv + q)
                nloc = nlpool.tile([128, K], F32, tag="nl")
                nc.vector.tensor_scalar(nloc[:], offp[:], qi_f[:], -1.0,
                                        OP.add, OP.mult)

                # Phase 0b: tent G build, transposed [q,u], then PE transpose
                gsb = gsbpool.tile([128, K, 128], F16, tag="gsb")
                masked = ci in MASKED_WINS
                wi = MASKED_WINS.index(ci) if masked else 0
                for k in range(K):
                    dab = dabpool.tile([128, 128], F32, tag="dab")
                    nc.scalar.activation(dab[:], iota_km[:, k, :], ACT.Abs,
                                         bias=nloc[:, k:k + 1])
                    tt = ttpool.tile([128, 128], F32, tag="tt")
                    nc.vector.tensor_scalar(tt[:], dab[:], 1.0, 0.0,
                                            OP.subtract, OP.min)
                    gt = gtpool.tile([128, 128], F16, tag="gt")
                    if masked:
                        c0 = 2 * K * wi + 2 * k
                        v1 = vnpool.tile([128, 1], F32, tag="v1")
                        nc.vector.tensor_scalar(
                            v1[:], nloc[:, k:k + 1], bnd_sb[:, c0:c0 + 1],
                            -1.0, OP.is_le, OP.mult)
                        vn = vnpool.tile([128, 1], F32, tag="vn")
                        nc.vector.scalar_tensor_tensor(
                            vn[:], nloc[:, k:k + 1], bnd_sb[:, c0 + 1:c0 + 2],
                            v1[:], OP.is_gt, OP.mult)
                        nc.gpsimd.tensor_scalar(gt[:], tt[:], vn[:], None,
                                                OP.mult)
                    else:
                        nc.gpsimd.tensor_scalar(gt[:], tt[:], -1.0, None,
                                                OP.mult)
                    trp = pstr.tile([128, 128], F16, tag="trp")
                    nc.tensor.transpose(trp[:], gt[:], ident[:])
                    nc.vector.tensor_copy(gsb[:, k, :], trp[:])

                # Phase 1: Y_k^T[t,o]
                ys = yspool.tile([128, K, COUT], F16, tag="ys")
                for k in range(K):
                    yp = psy.tile([128, COUT], F32, tag="yps")
                    for i in range(2):
                        lhs = x_sb[i][:, a0:a0 + 128]
                        nc.tensor.matmul(yp[:], lhs, w_sb[:, i, k, :],
                                         start=(i == 0), stop=(i == 1))
                    nc.scalar.copy(ys[:, k, :], yp[:])

                # Phase 2: out^T[q,o] = sum_k G_k^T @ Y_k^T
                ops = pso.tile([128, COUT], F32, tag="ops")
                for k in range(K):
                    nc.tensor.matmul(ops[:], gsb[:, k, :], ys[:, k, :],
                                     start=(k == 0), stop=(k == K - 1))
                osb = ospool.tile([128, COUT], F16, tag="o")
                nc.vector.tensor_add(osb[:], ops[:], bias_sb[:])
                rows = min(CHUNK, HALF - CHUNK * ci)
                nc.sync.dma_start(o_d[CHUNK * ci:CHUNK * ci + rows, :],
                                  osb[:rows, :])

    nc.finalize()
    return nc


def _get_runner():
    """Build the Bass program and a cached jitted PJRT callable once."""
    if "runner" in _CACHE:
        return _CACHE["runner"]
    nc = _build_nc()
    b2j.install_neuronx_cc_hook()

    partition_name = (nc.partition_id_tensor.name
                      if nc.partition_id_tensor else None)
    in_names, out_names, out_avals = [], [], []
    for alloc in nc.m.functions[0].allocations:
        if not isinstance(alloc, mybir.MemoryLocationSet):
            continue
        name = alloc.memorylocations[0].name
        if alloc.kind == "ExternalInput":
            if name != partition_name:
                in_names.append(name)
        elif alloc.kind == "ExternalOutput":
            out_names.append(name)
            out_avals.append(jax.core.ShapedArray(
                tuple(alloc.tensor_shape), mybir.dt.np(alloc.dtype)))
    n_params = len(in_names)
    n_outs = len(out_names)
    all_in_names = list(in_names) + list(out_names)
    if partition_name is not None:
        all_in_names.append(partition_name)
    donate = tuple(range(n_params, n_params + n_outs))

    def _body(*args):
        operands = list(args)
        if partition_name is not None:
            operands.append(b2j.partition_id_tensor())
        outs = b2j._bass_exec_p.bind(
            *operands,
            out_avals=tuple(out_avals),
            in_names=tuple(all_in_names),
            out_names=tuple(out_names),
            lowering_input_output_aliases=(),
            sim_require_finite=True,
            sim_require_nnan=True,
            nc=nc,
        )
        return tuple(outs)

    devices = jax.devices()[:NCORE]
    mesh = Mesh(np.asarray(devices), ("core",))
    sharded_names = {"xp", "bnd"}       # per-core inputs; rest replicated
    in_specs = tuple(
        PartitionSpec("core") if n in sharded_names else PartitionSpec()
        for n in in_names
    ) + (PartitionSpec("core"),) * n_outs
    out_specs = (PartitionSpec("core"),) * n_outs
    fn = jax.jit(
        shard_map(_body, mesh=mesh, in_specs=in_specs,
                  out_specs=out_specs, check_rep=False),
        donate_argnums=donate, keep_unused=True,
    )
    zsh = NamedSharding(mesh, PartitionSpec("core"))
    zshapes = [(NCORE * a.shape[0], *a.shape[1:]) for a in out_avals]
    zdtypes = [a.dtype for a in out_avals]
    zfn = jax.jit(
        lambda: tuple(jnp.zeros(s, d) for s, d in zip(zshapes, zdtypes)),
        out_shardings=(zsh,) * n_outs,
    )
    runner = (fn, zfn, in_names, out_names)
    _CACHE["runner"] = runner
    return runner


def _host_prep(x, weight, bias, offset_w, offset_b):
    """Pure data movement: slice/cast per-core inputs. No compute."""
    x = np.asarray(x, np.float32)
    weight = np.asarray(weight, np.float32)
    bias = np.asarray(bias, np.float32)
    offset_w = np.asarray(offset_w, np.float32)
    offset_b = np.asarray(offset_b, np.float32)

    xq = x.astype(np.float16)
    xp = np.zeros((NCORE, 2, 128, XPW), np.float16)
    bnd = np.empty((NCORE, 128, 4 * K), np.float32)
    crow = np.arange(K, dtype=np.float32) + HALO + offset_b
    for core in range(NCORE):
        b, half = divmod(core, 2)
        S = HALF * half
        lo, hi = S - HALO, S - HALO + XPW
        cl, ch = max(0, lo), min(L, hi)
        xp[core, :, :, cl - lo:ch - lo] = (
            xq[b, :, cl:ch].reshape(2, 128, ch - cl))
        for wi, win in enumerate(MASKED_WINS):
            band0 = S + win * CHUNK - HALO
            A = band0 + crow                       # valid: nloc <= A
            Bv = A - (L - 1)                       # and nloc > B
            bnd[core, :, 2 * K * wi + 0:2 * K * wi + 2 * K:2] = A
            bnd[core, :, 2 * K * wi + 1:2 * K * wi + 2 * K:2] = Bv

    wt = np.ascontiguousarray(
        weight.reshape(COUT, 2, 128, K).transpose(1, 3, 2, 0)).astype(np.float16)
    ow = np.ascontiguousarray(
        offset_w.transpose(1, 2, 0).reshape(2, 128, K, K).transpose(0, 2, 1, 3)
    ).astype(np.float16)
    ck = np.tile(crow, (128, 1)).astype(np.float32)

    return {
        "xp": xp.reshape(NCORE * 2, 128, XPW),
        "bnd": bnd.reshape(NCORE * 128, 4 * K),
        "wt": wt,
        "ow": ow,
        "ck": ck,
        "bias": bias.reshape(1, COUT),
    }


def _device_call(prepped):
    """The timed region: numpy in -> numpy out, one device roundtrip."""
    fn, zfn, in_names, out_names = _get_runner()
    zeros = zfn()                      # on-device, async
    args = [prepped[n] for n in in_names]
    outs = fn(*args, *zeros)
    return np.asarray(outs[0])


def _assemble(out_global):
    out = np.empty((B, COUT, L), np.float32)
    shards = out_global.reshape(NCORE, HALF, COUT)
    for core in range(NCORE):
        b, half = divmod(core, 2)
        S = HALF * half
        out[b, :, S:S + HALF] = shards[core].T.astype(np.float32)
    return out


def kernel(x, weight, bias, offset_w, offset_b):
    prepped = _host_prep(x, weight, bias, offset_w, offset_b)
    return _assemble(_device_call(prepped))


def kernel_timed(inputs, repeats=3):
    """Dev helper: returns (out, wall_times_s per device roundtrip)."""
    import time
    prepped = _host_prep(**inputs)
    _get_runner()
    times, og = [], None
    for _ in range(repeats):
        t0 = time.time()
        og = _device_call(prepped)
        times.append(time.time() - t0)
    return _assemble(og), times
